# revision 18
# baseline (speedup 1.0000x reference)
"""Trainium2 Bass kernel for the BuseE hyperbolic KG-embedding scorer.

Strategy (per core, 128 batch rows on the 128 SBUF partitions):
  The O(B*D) head-side work (entity/relation row lookup, expmap0/
  mobius/givens chain, s_h, sigma, per-row constant) runs on the host
  in f64 — it is 0.3% of the math. The device does the memory-bound
  part: fetching 131072 random 256B embedding rows per core and
  scoring them.

  Candidate rows are fetched with dma_gather (InstDMAGatherAnt) from a
  bf16 table [200000, 128] (256B rows = [emb(64) | 0-pad]). Each batch
  row's candidates are sorted ascending on the host; gather g covers
  sorted-rank columns [16g, 16g+16). Sorted column values concentrate
  around their quantiles, so a compile-time window base B_g with a
  32768-row span covers all partitions' values: indices fit int16 with
  no sharding and no overflow columns. Rare out-of-window candidates
  are clamped and their scores fixed up exactly in numpy.

  Gathers rotate over the 4 SWDGE queues; the Q7 cluster generates
  descriptors at ~2ns/idx aggregate (the hard throughput limit), so
  Pool runs nothing else. 16-col gathers (2048 idxs = 129 descs/DMA
  engine) double-buffer inside the per-queue descriptor ring; 32-col
  gathers overflow it and halve throughput.

  Per gather the vector engine computes dot = reduce(g_emb * h). The
  tail over [P,1024] uses host-precomputed per-candidate scalars
  c = tanh^2|x|, d = log(1-c) (shipped dense in sorted order):
      n2 = max(s_h - 2*dot + c, MIN)
      out = sig*d - ln(n2) + (MARGIN + bias_head + (1-sig)*ln(1-s_h))
  (tanh(|x|)/|x| ~ 1 to 2e-5 at this data scale, so dot needs no
  expmap rescale.)
  Host maps (b, n) -> sorted rank and reassembles with take_along_axis.
"""

import numpy as np
import ml_dtypes

import concourse.bacc as bacc
import concourse.bass as bass
import concourse.mybir as mybir
import concourse.tile as tile
from concourse import bass_utils

F32 = mybir.dt.float32
BF16 = mybir.dt.bfloat16
I16 = mybir.dt.int16
AX = mybir.AxisListType
OP = mybir.AluOpType
AF = mybir.ActivationFunctionType

MIN_NORM = 1e-15
MARGIN = 9.0
N_ENT, N_REL, D = 200000, 500, 64
B, NCAND = 1024, 1024
NCORES = 8
P = 128                   # batch rows per core == partitions
EW = 128                  # bf16 elems per table row (256B)

GCH = 16                  # sorted-rank columns per gather
NG = 53                   # SWDGE gathers per core (cols 0 .. NG*GCH-1)
NSW = NG * GCH            # 848 sorted cols via SWDGE
NIND = NCAND - NSW        # 176 sorted cols via HWDGE [P,1] indirects
NCHI = NIND // GCH        # indirect chunks of 16 cols
WIN = 32768               # int16 window rows
NQ_SWDGE = 4
NI = GCH * P              # idxs per gather
IC = NI // 16             # int16 idx columns per gather

# compile-time window base per gather: centered on the mid-column quantile
GBASE = [
    int(np.clip(round(N_ENT * (g * GCH + GCH // 2) / NCAND) - WIN // 2,
                0, N_ENT - WIN))
    for g in range(NG)
]

_CACHE: dict = {}


def _patch_tile_lane_assignment():
    """Make Tile's DMASW completion-lane rotation queue-aware.

    Tile round-robins Pool-engine DMAs over 8 DMASW lanes ignoring the
    SWDGE queue_num; the SWDGE ucode locks each completion sem lane to
    one queue, so multi-queue kernels hit cross-queue lane collisions.
    Give each queue a fixed pair of lanes: queue q -> lanes {2q, 2q+1}.
    """
    import inspect
    import textwrap
    from concourse import tile_sem_assignment as tsa

    if getattr(tsa, "_lane_patch_done", False):
        return
    src = inspect.getsource(tsa.TileClockTick._assign_tick)
    old = """            if engine == mybir.EngineType.Pool:
                inst_proc_idx = PROC_NAME_TO_IDX[f"DMASW{self.next_sw_dma_idx}"]
                self.next_sw_dma_idx = (self.next_sw_dma_idx + 1) % self.swdge_sem_count"""
    new = """            if engine == mybir.EngineType.Pool:
                if type(inst).__name__ == "InstDMAGatherAnt":
                    _lane = int(getattr(inst, "queue_num", 0) or 0)
                else:
                    _cnt = getattr(self, "_hw_lane_counter", 0)
                    self._hw_lane_counter = _cnt + 1
                    _lane = 4 + (_cnt % 4)
                inst_proc_idx = PROC_NAME_TO_IDX[f"DMASW{_lane}"]
                self.next_sw_dma_idx = (self.next_sw_dma_idx + 1) % self.swdge_sem_count"""
    assert old in textwrap.dedent(src) or old in src, "tile lane patch anchor missing"
    patched = src.replace(old, new)
    ns = dict(vars(tsa))
    exec(textwrap.dedent(patched), ns)
    tsa.TileClockTick._assign_tick = ns["_assign_tick"]
    tsa._lane_patch_done = True


def _build(with_bias):
    _patch_tile_lane_assignment()
    nc = bacc.Bacc(
        "TRN2",
        target_bir_lowering=False,
        debug=False,
        enable_asserts=False,
        num_devices=NCORES,
        num_swdge_queues=NQ_SWDGE,
    )
    TB = nc.dram_tensor("tab_bf", [N_ENT, EW], BF16, kind="ExternalInput")
    GI = nc.dram_tensor("gidx", [P, NG * IC], I16, kind="ExternalInput")
    OFI = nc.dram_tensor("of_idx", [P, NIND], mybir.dt.int32, kind="ExternalInput")
    HBF = nc.dram_tensor("h_bf", [P, D], BF16, kind="ExternalInput")
    ROWC = nc.dram_tensor("rowc", [P, 3], F32, kind="ExternalInput")  # s_h|sig|c_b
    CA = nc.dram_tensor("c_all", [P, NCAND], BF16, kind="ExternalInput")
    DA = nc.dram_tensor("d_all", [P, NCAND], BF16, kind="ExternalInput")
    BT = (nc.dram_tensor("bt_all", [P, NCAND], BF16, kind="ExternalInput")
          if with_bias else None)
    OUT = nc.dram_tensor("out", [P, NCAND], F32, kind="ExternalOutput")

    with tile.TileContext(nc) as tc:
        with (
            tc.tile_pool(name="small", bufs=1) as sp,
            tc.tile_pool(name="big", bufs=2) as bp,
        ):
            # first gather's indices load before everything else
            gidx0 = bp.tile([P, IC], I16, tag="gidx", name="gidx0", bufs=8)
            nc.sync.dma_start(gidx0[:], GI[:, 0:IC])
            hbf = sp.tile([P, D], BF16)
            nc.sync.dma_start(hbf[:], HBF[:])
            rowc = sp.tile([P, 3], F32)
            nc.sync.dma_start(rowc[:], ROWC[:])
            ofi = sp.tile([P, NIND], mybir.dt.int32)
            nc.sync.dma_start(ofi[:], OFI[:])
            s_h = rowc[:, 0:1]
            sig = rowc[:, 1:2]
            c_b = rowc[:, 2:3]

            out_sb = sp.tile([P, NCAND], F32)
            dot_all = sp.tile([P, NCAND], F32)

            h_b = hbf[:].rearrange("p (one d) -> p one d", one=1).to_broadcast(
                [P, GCH, D]
            )

            def chunk_math(g3, ksl, tag, g):
                g64 = g3[:, :, 0:D]
                pr = bp.tile([P, GCH * D], BF16, tag="pr", name=f"pr{tag}{g}", bufs=4)
                pr3 = pr[:].rearrange("p (n d) -> p n d", d=D)
                nc.vector.tensor_tensor(pr3, g64, h_b, op=OP.mult)
                nc.vector.tensor_reduce(dot_all[:, ksl], pr3, axis=AX.X, op=OP.add)

            # ---- candidate fetch: SWDGE gathers + interleaved HW indirects ----
            ind_next = 0            # next indirect column to issue (0..NIND)
            ind_tile = None
            ind_t3 = None

            def issue_indirects(k):
                nonlocal ind_next, ind_tile, ind_t3
                for _ in range(k):
                    if ind_next >= NIND:
                        return
                    j = ind_next % GCH
                    if j == 0:
                        ind_tile = bp.tile([P, GCH * EW], BF16, tag="gi",
                                           name=f"gi{ind_next // GCH}", bufs=4)
                        ind_t3 = ind_tile[:].rearrange("p (n d) -> p n d", d=EW)
                    nc.gpsimd.indirect_dma_start(
                        out=ind_t3[:, j, :], out_offset=None, in_=TB[:],
                        in_offset=bass.IndirectOffsetOnAxis(
                            ap=ofi[:, ind_next:ind_next + 1], axis=0),
                    )
                    ind_next += 1
                    if j == GCH - 1:
                        ch = ind_next // GCH - 1
                        ksl = slice(NSW + ch * GCH, NSW + (ch + 1) * GCH)
                        chunk_math(ind_t3, ksl, "i", ch)

            for g in range(NG):
                q = g % NQ_SWDGE
                ksl = slice(g * GCH, (g + 1) * GCH)
                if g == 0:
                    gidx_t = gidx0
                else:
                    gidx_t = bp.tile([P, IC], I16, tag="gidx", name=f"gidx{g}", bufs=8)
                    nc.sync.dma_start(gidx_t[:], GI[:, g * IC:(g + 1) * IC])
                gt = bp.tile([P, GCH * EW], BF16, tag="g", name=f"g{g}", bufs=8)
                g3 = gt[:].rearrange("p (n d) -> p n d", d=EW)
                nc.gpsimd.dma_gather(
                    out_ap=g3,
                    in_ap=TB[GBASE[g]:GBASE[g] + WIN, :],
                    idxs_ap=gidx_t[:],
                    num_idxs=NI,
                    num_idxs_reg=NI,
                    elem_size=EW,
                    single_packet=False,
                    queue_num=q,
                )
                issue_indirects(4 if g < NIND - 3 * NG else 3)
                chunk_math(g3, ksl, "g", g)
            issue_indirects(NIND - ind_next)

            # tail inputs load late on sync so gidx DMAs go first
            c_all = sp.tile([P, NCAND], BF16)
            nc.sync.dma_start(c_all[:], CA[:])
            d_all = sp.tile([P, NCAND], BF16)
            nc.sync.dma_start(d_all[:], DA[:])
            bt_all = None
            if with_bias:
                bt_all = sp.tile([P, NCAND], BF16)
                nc.sync.dma_start(bt_all[:], BT[:])

            # ---- batched tail over [P, NCAND] ----
            n2 = sp.tile([P, NCAND], F32)
            nc.vector.scalar_tensor_tensor(
                n2[:], dot_all[:], -2.0, c_all[:], op0=OP.mult, op1=OP.add
            )
            nc.vector.tensor_scalar_add(n2[:], n2[:], s_h)
            nc.vector.tensor_scalar_max(n2[:], n2[:], MIN_NORM)
            lnum = sp.tile([P, NCAND], F32)
            nc.scalar.activation(lnum[:], n2[:], AF.Ln)
            res = sp.tile([P, NCAND], F32)
            nc.vector.scalar_tensor_tensor(
                res[:], d_all[:], sig, lnum[:], op0=OP.mult, op1=OP.subtract
            )
            if with_bias:
                nc.vector.scalar_tensor_tensor(
                    out_sb[:], res[:], c_b, bt_all[:], op0=OP.add, op1=OP.add
                )
            else:
                nc.vector.tensor_scalar_add(out_sb[:], res[:], c_b)

            nc.sync.dma_start(OUT[:], out_sb[:])

    nc.compile()
    return nc


def get_module(with_bias=False):
    key = ("nc", bool(with_bias))
    if key not in _CACHE:
        _CACHE[key] = _build(bool(with_bias))
    return _CACHE[key]


def _np_head_chain(u_idx, r_idx, emb, rel_diag, rb1, rb2):
    """Transformed heads [B, D] in f64, mirroring the reference chain."""
    def expmap0(u):
        un = np.maximum(np.linalg.norm(u, axis=-1, keepdims=True), MIN_NORM)
        return np.tanh(un) * u / un

    def mobius_add(x, y):
        x2 = np.sum(x * x, -1, keepdims=True)
        y2 = np.sum(y * y, -1, keepdims=True)
        xy = np.sum(x * y, -1, keepdims=True)
        num = (1.0 + 2.0 * xy + y2) * x + (1.0 - x2) * y
        den = 1.0 + 2.0 * xy + x2 * y2
        return num / np.maximum(den, MIN_NORM)

    def givens(r, x):
        g = r.reshape(r.shape[:-1] + (-1, 2))
        g = g / np.maximum(np.linalg.norm(g, axis=-1, keepdims=True), MIN_NORM)
        xp = x.reshape(x.shape[:-1] + (-1, 2))
        out = np.stack(
            [g[..., 0] * xp[..., 0] - g[..., 1] * xp[..., 1],
             g[..., 1] * xp[..., 0] + g[..., 0] * xp[..., 1]], axis=-1)
        return out.reshape(x.shape)

    head = expmap0(emb[u_idx])
    head = mobius_add(head, expmap0(rb1[r_idx]))
    head = givens(rel_diag[r_idx], head)
    head = mobius_add(head, expmap0(rb2[r_idx]))
    return head


def make_in_maps(u_idx, r_idx, v_idx, emb_entity, rel_diag, relation_bias_1,
                 relation_bias_2, bias_head, bias_tail, sigma):
    emb = np.ascontiguousarray(np.asarray(emb_entity, dtype=np.float32))
    bt = np.asarray(bias_tail, dtype=np.float32)
    ui64 = np.asarray(u_idx).astype(np.int64).reshape(B)
    ri64 = np.asarray(r_idx).astype(np.int64).reshape(B)
    vi = np.asarray(v_idx).astype(np.int64).reshape(B, NCAND)

    # per-entity tail scalars in f64: c = tanh^2|x|, d = log(1 - c)
    emb64 = emb.astype(np.float64)
    s = np.sum(emb64 ** 2, axis=1)
    un = np.maximum(np.sqrt(s), MIN_NORM)
    th = np.tanh(un)
    c = th * th
    dcol = np.log(np.maximum(1.0 - c, MIN_NORM))
    tab = np.zeros((N_ENT, EW), dtype=ml_dtypes.bfloat16)
    tab[:, 0:D] = emb.astype(ml_dtypes.bfloat16)

    # host-side head chain + per-row constants
    heads = _np_head_chain(ui64, ri64, emb64,
                           np.asarray(rel_diag, np.float64),
                           np.asarray(relation_bias_1, np.float64),
                           np.asarray(relation_bias_2, np.float64))
    s_h = np.sum(heads * heads, axis=-1)
    sg = 1.0 / (1.0 + np.exp(-np.asarray(sigma, np.float64)[ri64]))
    cb = (MARGIN + np.asarray(bias_head, np.float64)[ui64]
          + (1.0 - sg) * np.log(np.maximum(1.0 - s_h, MIN_NORM)))
    h_bf = heads.astype(ml_dtypes.bfloat16)
    rowc = np.stack([s_h, sg, cb], axis=1).astype(np.float32)   # [B, 3]

    has_bias = bool(np.any(bt))

    order = np.argsort(vi, axis=1, kind="stable")        # [B, NCAND]
    ranks = np.empty_like(order, dtype=np.int64)
    np.put_along_axis(ranks, order, np.arange(NCAND, dtype=np.int64)[None, :], axis=1)
    vs = np.take_along_axis(vi, order, axis=1)           # sorted values

    bases = np.repeat(np.asarray(GBASE, dtype=np.int64), GCH)[None, :]  # [1, NSW]
    loc = vs[:, :NSW] - bases                            # window-local
    viol = np.zeros((B, NCAND), dtype=bool)
    viol[:, :NSW] = (loc < 0) | (loc > WIN - 1)
    loc_cl = np.clip(loc, 0, WIN - 1).astype(np.int16)

    c_bf = c.astype(ml_dtypes.bfloat16)
    d_bf = dcol.astype(ml_dtypes.bfloat16)
    bt_bf = bt.astype(ml_dtypes.bfloat16)
    in_maps = []
    aux_ranks = []
    for cidx in range(NCORES):
        sl = slice(cidx * P, (cidx + 1) * P)
        lc = loc_cl[sl]                                  # [P, NSW] int16
        parts = []
        for g in range(NG):
            st = lc[:, g * GCH:(g + 1) * GCH]            # [P, GCH]
            stream = st.T.ravel()                        # i = c*128 + p
            wrapped = stream.reshape(-1, 16).T           # [16, NI/16]
            parts.append(np.tile(wrapped, (8, 1)))       # [128, NI/16]
        gidx = np.ascontiguousarray(np.concatenate(parts, axis=1))
        assert gidx.shape == (P, NG * IC)
        vs_c = vs[sl]                                    # sorted entity ids
        in_map = {
            "tab_bf": tab,
            "gidx": gidx,
            "of_idx": np.ascontiguousarray(vs_c[:, NSW:].astype(np.int32)),
            "h_bf": np.ascontiguousarray(h_bf[sl]),
            "rowc": np.ascontiguousarray(rowc[sl]),
            "c_all": np.ascontiguousarray(c_bf[vs_c]),
            "d_all": np.ascontiguousarray(d_bf[vs_c]),
        }
        if has_bias:
            in_map["bt_all"] = np.ascontiguousarray(bt_bf[vs_c])
        in_maps.append(in_map)
        aux_ranks.append(ranks[sl])

    # exact host fixup values for window-violating candidates
    fix = None
    nviol = int(viol.sum())
    if nviol:
        vb, vc = np.nonzero(viol)                        # batch row, sorted col
        v_ent = vs[vb, vc]                               # entity ids
        hb = heads[vb]                                   # [K, D]
        x = emb64[v_ent]
        unx = np.maximum(np.linalg.norm(x, axis=-1, keepdims=True), MIN_NORM)
        t = np.tanh(unx) * x / unx                       # expmap0(tail)
        n2 = np.sum((hb - t) ** 2, axis=-1)
        s_hb = np.sum(hb * hb, axis=-1)
        s_t = np.sum(t * t, axis=-1)
        d_tail = np.log(np.maximum(n2, MIN_NORM) / np.maximum(1.0 - s_t, MIN_NORM))
        d_head = np.log(np.maximum(n2, MIN_NORM) / np.maximum(1.0 - s_hb, MIN_NORM))
        sgv = sg[vb]
        dist = sgv * d_tail + (1.0 - sgv) * d_head
        val = (MARGIN - dist
               + np.asarray(bias_head, np.float64)[ui64[vb]]
               + np.asarray(bias_tail, np.float64)[v_ent])
        fix = (vb, vc, val.astype(np.float32))
    return in_maps, (aux_ranks, fix), has_bias


def assemble(results, aux):
    aux_ranks, fix = aux
    sorted_scores = np.concatenate(
        [np.asarray(results[c]["out"]) for c in range(NCORES)], axis=0
    )                                                    # [B, NCAND] sorted cols
    if fix is not None:
        vb, vc, val = fix
        sorted_scores[vb, vc] = val
    ranks = np.concatenate(aux_ranks, axis=0)
    return np.take_along_axis(sorted_scores, ranks, axis=1).astype(np.float32)


def kernel(**inputs) -> np.ndarray:
    in_maps, aux, has_bias = make_in_maps(**inputs)
    nc = get_module(has_bias)
    res = bass_utils.run_bass_kernel_spmd(
        nc, in_maps, core_ids=list(range(NCORES))
    )
    return assemble(res.results, aux)


# revision 20
# speedup vs baseline: 3.5286x; 3.5286x over previous
"""Trainium2 Bass kernel for the BuseE hyperbolic KG-embedding scorer.

Strategy (per core, 128 batch rows on the 128 SBUF partitions):
  The O(B*D) head-side work (entity/relation row lookup, expmap0/
  mobius/givens chain, s_h, sigma, per-row constant) runs on the host
  in f64 — it is 0.3% of the math. The device does the memory-bound
  part: fetching 131072 random 256B embedding rows per core and
  scoring them.

  Candidate rows are fetched with dma_gather (InstDMAGatherAnt) from a
  bf16 table [200000, 128] (256B rows = [emb(64) | 0-pad]). Each batch
  row's candidates are sorted ascending on the host; gather g covers
  sorted-rank columns [16g, 16g+16). Sorted column values concentrate
  around their quantiles, so a compile-time window base B_g with a
  32768-row span covers all partitions' values: indices fit int16 with
  no sharding and no overflow columns. Rare out-of-window candidates
  are clamped and their scores fixed up exactly in numpy.

  Gathers rotate over the 4 SWDGE queues; the Q7 cluster generates
  descriptors at ~2ns/idx aggregate (the hard throughput limit), so
  Pool runs nothing else. 16-col gathers (2048 idxs = 129 descs/DMA
  engine) double-buffer inside the per-queue descriptor ring; 32-col
  gathers overflow it and halve throughput.

  Per gather the vector engine computes dot = reduce(g_emb * h). The
  tail over [P,1024] uses host-precomputed per-candidate scalars
  c = tanh^2|x|, d = log(1-c) (shipped dense in sorted order):
      n2 = max(s_h - 2*dot + c, MIN)
      out = sig*d - ln(n2) + (MARGIN + bias_head + (1-sig)*ln(1-s_h))
  (tanh(|x|)/|x| ~ 1 to 2e-5 at this data scale, so dot needs no
  expmap rescale.)
  Host maps (b, n) -> sorted rank and reassembles with take_along_axis.
"""

import numpy as np
import ml_dtypes

import concourse.bacc as bacc
import concourse.bass as bass
import concourse.mybir as mybir
import concourse.tile as tile
from concourse import bass_utils

F32 = mybir.dt.float32
BF16 = mybir.dt.bfloat16
I16 = mybir.dt.int16
AX = mybir.AxisListType
OP = mybir.AluOpType
AF = mybir.ActivationFunctionType

MIN_NORM = 1e-15
MARGIN = 9.0
N_ENT, N_REL, D = 200000, 500, 64
B, NCAND = 1024, 1024
NCORES = 8
P = 128                   # batch rows per core == partitions
EW = 128                  # bf16 elems per table row (256B)

GCH = 16                  # sorted-rank columns per gather
NG = NCAND // GCH         # gathers per core
WIN = 32768               # int16 window rows
NQ_SWDGE = 4
NI = GCH * P              # idxs per gather
IC = NI // 16             # int16 idx columns per gather

# compile-time window base per gather: centered on the mid-column quantile
GBASE = [
    int(np.clip(round(N_ENT * (g * GCH + GCH // 2) / NCAND) - WIN // 2,
                0, N_ENT - WIN))
    for g in range(NG)
]

_CACHE: dict = {}


def _patch_tile_lane_assignment():
    """Make Tile's DMASW completion-lane rotation queue-aware.

    Tile round-robins Pool-engine DMAs over 8 DMASW lanes ignoring the
    SWDGE queue_num; the SWDGE ucode locks each completion sem lane to
    one queue, so multi-queue kernels hit cross-queue lane collisions.
    Give each queue a fixed pair of lanes: queue q -> lanes {2q, 2q+1}.
    """
    import inspect
    import textwrap
    from concourse import tile_sem_assignment as tsa

    if getattr(tsa, "_lane_patch_done", False):
        return
    src = inspect.getsource(tsa.TileClockTick._assign_tick)
    old = """            if engine == mybir.EngineType.Pool:
                inst_proc_idx = PROC_NAME_TO_IDX[f"DMASW{self.next_sw_dma_idx}"]
                self.next_sw_dma_idx = (self.next_sw_dma_idx + 1) % self.swdge_sem_count"""
    new = """            if engine == mybir.EngineType.Pool:
                _q = int(getattr(inst, "queue_num", 0) or 0)
                _cnt = getattr(self, "_q_lane_counter", None)
                if _cnt is None:
                    _cnt = self._q_lane_counter = {}
                _c = _cnt.get(_q, 0)
                _cnt[_q] = _c + 1
                _lane = (2 * _q + (_c % 2)) % self.swdge_sem_count
                inst_proc_idx = PROC_NAME_TO_IDX[f"DMASW{_lane}"]
                self.next_sw_dma_idx = (self.next_sw_dma_idx + 1) % self.swdge_sem_count"""
    assert old in textwrap.dedent(src) or old in src, "tile lane patch anchor missing"
    patched = src.replace(old, new)
    ns = dict(vars(tsa))
    exec(textwrap.dedent(patched), ns)
    tsa.TileClockTick._assign_tick = ns["_assign_tick"]
    tsa._lane_patch_done = True


def _build(with_bias):
    _patch_tile_lane_assignment()
    nc = bacc.Bacc(
        "TRN2",
        target_bir_lowering=False,
        debug=False,
        enable_asserts=False,
        num_devices=NCORES,
        num_swdge_queues=NQ_SWDGE,
    )
    TB = nc.dram_tensor("tab_bf", [N_ENT, EW], BF16, kind="ExternalInput")
    GI = nc.dram_tensor("gidx", [P, NG * IC], I16, kind="ExternalInput")
    HBF = nc.dram_tensor("h_bf", [P, D], BF16, kind="ExternalInput")
    ROWC = nc.dram_tensor("rowc", [P, 3], F32, kind="ExternalInput")  # s_h|sig|c_b
    CA = nc.dram_tensor("c_all", [P, NCAND], BF16, kind="ExternalInput")
    DA = nc.dram_tensor("d_all", [P, NCAND], BF16, kind="ExternalInput")
    BT = (nc.dram_tensor("bt_all", [P, NCAND], BF16, kind="ExternalInput")
          if with_bias else None)
    OUT = nc.dram_tensor("out", [P, NCAND], F32, kind="ExternalOutput")

    with tile.TileContext(nc) as tc:
        with (
            tc.tile_pool(name="small", bufs=1) as sp,
            tc.tile_pool(name="big", bufs=2) as bp,
        ):
            # first gather's indices load before everything else
            gidx0 = bp.tile([P, IC], I16, tag="gidx", name="gidx0", bufs=8)
            nc.sync.dma_start(gidx0[:], GI[:, 0:IC])
            hbf = sp.tile([P, D], BF16)
            nc.sync.dma_start(hbf[:], HBF[:])
            rowc = sp.tile([P, 3], F32)
            nc.sync.dma_start(rowc[:], ROWC[:])
            s_h = rowc[:, 0:1]
            sig = rowc[:, 1:2]
            c_b = rowc[:, 2:3]
            c_all = sp.tile([P, NCAND], BF16)
            nc.sync.dma_start(c_all[:], CA[:])
            d_all = sp.tile([P, NCAND], BF16)
            nc.sync.dma_start(d_all[:], DA[:])
            bt_all = None
            if with_bias:
                bt_all = sp.tile([P, NCAND], BF16)
                nc.sync.dma_start(bt_all[:], BT[:])

            out_sb = sp.tile([P, NCAND], F32)
            dot_all = sp.tile([P, NCAND], F32)
            n2 = sp.tile([P, NCAND], F32)
            lnum = sp.tile([P, NCAND], F32)
            res = sp.tile([P, NCAND], F32)

            def tail_half(h):
                hs = slice(h * (NCAND // 2), (h + 1) * (NCAND // 2))
                nc.vector.scalar_tensor_tensor(
                    n2[:, hs], dot_all[:, hs], -2.0, c_all[:, hs],
                    op0=OP.mult, op1=OP.add
                )
                nc.vector.tensor_scalar_add(n2[:, hs], n2[:, hs], s_h)
                nc.vector.tensor_scalar_max(n2[:, hs], n2[:, hs], MIN_NORM)
                nc.scalar.activation(lnum[:, hs], n2[:, hs], AF.Ln)
                nc.vector.scalar_tensor_tensor(
                    res[:, hs], d_all[:, hs], sig, lnum[:, hs],
                    op0=OP.mult, op1=OP.subtract
                )
                if with_bias:
                    nc.vector.scalar_tensor_tensor(
                        out_sb[:, hs], res[:, hs], c_b, bt_all[:, hs],
                        op0=OP.add, op1=OP.add
                    )
                else:
                    nc.vector.tensor_scalar_add(out_sb[:, hs], res[:, hs], c_b)
                nc.sync.dma_start(OUT[:, hs], out_sb[:, hs])

            # ---- candidate gathers: fetch + dot only ----
            for g in range(NG):
                q = g % NQ_SWDGE
                ksl = slice(g * GCH, (g + 1) * GCH)
                if g == 0:
                    gidx_t = gidx0
                else:
                    gidx_t = bp.tile([P, IC], I16, tag="gidx", name=f"gidx{g}", bufs=8)
                    nc.sync.dma_start(gidx_t[:], GI[:, g * IC:(g + 1) * IC])
                gt = bp.tile([P, GCH * EW], BF16, tag="g", name=f"g{g}", bufs=8)
                g3 = gt[:].rearrange("p (n d) -> p n d", d=EW)
                nc.gpsimd.dma_gather(
                    out_ap=g3,
                    in_ap=TB[GBASE[g]:GBASE[g] + WIN, :],
                    idxs_ap=gidx_t[:],
                    num_idxs=NI,
                    num_idxs_reg=NI,
                    elem_size=EW,
                    single_packet=False,
                    queue_num=q,
                )
                g64 = g3[:, :, 0:D]
                h_b = hbf[:].rearrange("p (one d) -> p one d", one=1).to_broadcast(
                    [P, GCH, D]
                )
                pr = bp.tile([P, GCH * D], BF16, tag="pr", name=f"pr{g}", bufs=4)
                pr3 = pr[:].rearrange("p (n d) -> p n d", d=D)
                nc.vector.tensor_tensor(pr3, g64, h_b, op=OP.mult)
                nc.vector.tensor_reduce(dot_all[:, ksl], pr3, axis=AX.X, op=OP.add)
                if g == NG // 2 + 3:
                    tail_half(0)
            tail_half(1)

    nc.compile()
    return nc


def get_module(with_bias=False):
    key = ("nc", bool(with_bias))
    if key not in _CACHE:
        _CACHE[key] = _build(bool(with_bias))
    return _CACHE[key]


def _np_head_chain(u_idx, r_idx, emb, rel_diag, rb1, rb2):
    """Transformed heads [B, D] in f64, mirroring the reference chain."""
    def expmap0(u):
        un = np.maximum(np.linalg.norm(u, axis=-1, keepdims=True), MIN_NORM)
        return np.tanh(un) * u / un

    def mobius_add(x, y):
        x2 = np.sum(x * x, -1, keepdims=True)
        y2 = np.sum(y * y, -1, keepdims=True)
        xy = np.sum(x * y, -1, keepdims=True)
        num = (1.0 + 2.0 * xy + y2) * x + (1.0 - x2) * y
        den = 1.0 + 2.0 * xy + x2 * y2
        return num / np.maximum(den, MIN_NORM)

    def givens(r, x):
        g = r.reshape(r.shape[:-1] + (-1, 2))
        g = g / np.maximum(np.linalg.norm(g, axis=-1, keepdims=True), MIN_NORM)
        xp = x.reshape(x.shape[:-1] + (-1, 2))
        out = np.stack(
            [g[..., 0] * xp[..., 0] - g[..., 1] * xp[..., 1],
             g[..., 1] * xp[..., 0] + g[..., 0] * xp[..., 1]], axis=-1)
        return out.reshape(x.shape)

    head = expmap0(emb[u_idx])
    head = mobius_add(head, expmap0(rb1[r_idx]))
    head = givens(rel_diag[r_idx], head)
    head = mobius_add(head, expmap0(rb2[r_idx]))
    return head


def make_in_maps(u_idx, r_idx, v_idx, emb_entity, rel_diag, relation_bias_1,
                 relation_bias_2, bias_head, bias_tail, sigma):
    emb = np.ascontiguousarray(np.asarray(emb_entity, dtype=np.float32))
    bt = np.asarray(bias_tail, dtype=np.float32)
    ui64 = np.asarray(u_idx).astype(np.int64).reshape(B)
    ri64 = np.asarray(r_idx).astype(np.int64).reshape(B)
    vi = np.asarray(v_idx).astype(np.int64).reshape(B, NCAND)

    # per-entity tail scalars in f64: c = tanh^2|x|, d = log(1 - c)
    emb64 = emb.astype(np.float64)
    s = np.sum(emb64 ** 2, axis=1)
    un = np.maximum(np.sqrt(s), MIN_NORM)
    th = np.tanh(un)
    c = th * th
    dcol = np.log(np.maximum(1.0 - c, MIN_NORM))
    tab = np.zeros((N_ENT, EW), dtype=ml_dtypes.bfloat16)
    tab[:, 0:D] = emb.astype(ml_dtypes.bfloat16)

    # host-side head chain + per-row constants
    heads = _np_head_chain(ui64, ri64, emb64,
                           np.asarray(rel_diag, np.float64),
                           np.asarray(relation_bias_1, np.float64),
                           np.asarray(relation_bias_2, np.float64))
    s_h = np.sum(heads * heads, axis=-1)
    sg = 1.0 / (1.0 + np.exp(-np.asarray(sigma, np.float64)[ri64]))
    cb = (MARGIN + np.asarray(bias_head, np.float64)[ui64]
          + (1.0 - sg) * np.log(np.maximum(1.0 - s_h, MIN_NORM)))
    h_bf = heads.astype(ml_dtypes.bfloat16)
    rowc = np.stack([s_h, sg, cb], axis=1).astype(np.float32)   # [B, 3]

    has_bias = bool(np.any(bt))

    order = np.argsort(vi, axis=1, kind="stable")        # [B, NCAND]
    ranks = np.empty_like(order, dtype=np.int64)
    np.put_along_axis(ranks, order, np.arange(NCAND, dtype=np.int64)[None, :], axis=1)
    vs = np.take_along_axis(vi, order, axis=1)           # sorted values

    bases = np.repeat(np.asarray(GBASE, dtype=np.int64), GCH)[None, :]  # [1, NCAND]
    loc = vs - bases                                     # window-local
    viol = (loc < 0) | (loc > WIN - 1)                   # [B, NCAND] on sorted cols
    loc_cl = np.clip(loc, 0, WIN - 1).astype(np.int16)

    c_bf = c.astype(ml_dtypes.bfloat16)
    d_bf = dcol.astype(ml_dtypes.bfloat16)
    bt_bf = bt.astype(ml_dtypes.bfloat16)
    in_maps = []
    aux_ranks = []
    for cidx in range(NCORES):
        sl = slice(cidx * P, (cidx + 1) * P)
        lc = loc_cl[sl]                                  # [P, NCAND] int16
        parts = []
        for g in range(NG):
            st = lc[:, g * GCH:(g + 1) * GCH]            # [P, GCH]
            stream = st.T.ravel()                        # i = c*128 + p
            wrapped = stream.reshape(-1, 16).T           # [16, NI/16]
            parts.append(np.tile(wrapped, (8, 1)))       # [128, NI/16]
        gidx = np.ascontiguousarray(np.concatenate(parts, axis=1))
        assert gidx.shape == (P, NG * IC)
        vs_c = vs[sl]                                    # sorted entity ids
        in_map = {
            "tab_bf": tab,
            "gidx": gidx,
            "h_bf": np.ascontiguousarray(h_bf[sl]),
            "rowc": np.ascontiguousarray(rowc[sl]),
            "c_all": np.ascontiguousarray(c_bf[vs_c]),
            "d_all": np.ascontiguousarray(d_bf[vs_c]),
        }
        if has_bias:
            in_map["bt_all"] = np.ascontiguousarray(bt_bf[vs_c])
        in_maps.append(in_map)
        aux_ranks.append(ranks[sl])

    # exact host fixup values for window-violating candidates
    fix = None
    nviol = int(viol.sum())
    if nviol:
        vb, vc = np.nonzero(viol)                        # batch row, sorted col
        v_ent = vs[vb, vc]                               # entity ids
        hb = heads[vb]                                   # [K, D]
        x = emb64[v_ent]
        unx = np.maximum(np.linalg.norm(x, axis=-1, keepdims=True), MIN_NORM)
        t = np.tanh(unx) * x / unx                       # expmap0(tail)
        n2 = np.sum((hb - t) ** 2, axis=-1)
        s_hb = np.sum(hb * hb, axis=-1)
        s_t = np.sum(t * t, axis=-1)
        d_tail = np.log(np.maximum(n2, MIN_NORM) / np.maximum(1.0 - s_t, MIN_NORM))
        d_head = np.log(np.maximum(n2, MIN_NORM) / np.maximum(1.0 - s_hb, MIN_NORM))
        sgv = sg[vb]
        dist = sgv * d_tail + (1.0 - sgv) * d_head
        val = (MARGIN - dist
               + np.asarray(bias_head, np.float64)[ui64[vb]]
               + np.asarray(bias_tail, np.float64)[v_ent])
        fix = (vb, vc, val.astype(np.float32))
    return in_maps, (aux_ranks, fix), has_bias


def assemble(results, aux):
    aux_ranks, fix = aux
    sorted_scores = np.concatenate(
        [np.asarray(results[c]["out"]) for c in range(NCORES)], axis=0
    )                                                    # [B, NCAND] sorted cols
    if fix is not None:
        vb, vc, val = fix
        sorted_scores[vb, vc] = val
    ranks = np.concatenate(aux_ranks, axis=0)
    return np.take_along_axis(sorted_scores, ranks, axis=1).astype(np.float32)


def kernel(**inputs) -> np.ndarray:
    in_maps, aux, has_bias = make_in_maps(**inputs)
    nc = get_module(has_bias)
    res = bass_utils.run_bass_kernel_spmd(
        nc, in_maps, core_ids=list(range(NCORES))
    )
    return assemble(res.results, aux)


# revision 23
# speedup vs baseline: 3.6055x; 1.0218x over previous
"""Trainium2 Bass kernel for the BuseE hyperbolic KG-embedding scorer.

Strategy (per core, 128 batch rows on the 128 SBUF partitions):
  The O(B*D) head-side work (entity/relation row lookup, expmap0/
  mobius/givens chain, s_h, sigma, per-row constant) runs on the host
  in f64 — it is 0.3% of the math. The device does the memory-bound
  part: fetching 131072 random 256B embedding rows per core and
  scoring them.

  Candidate rows are fetched with dma_gather (InstDMAGatherAnt) from a
  bf16 table [200000, 128] (256B rows = [emb(64) | 0-pad]). Each batch
  row's candidates are sorted ascending on the host; gather g covers
  sorted-rank columns [16g, 16g+16). Sorted column values concentrate
  around their quantiles, so a compile-time window base B_g with a
  32768-row span covers all partitions' values: indices fit int16 with
  no sharding and no overflow columns. Rare out-of-window candidates
  are clamped and their scores fixed up exactly in numpy.

  Gathers rotate over the 4 SWDGE queues; the Q7 cluster generates
  descriptors at ~2ns/idx aggregate (the hard throughput limit), so
  Pool runs nothing else. 16-col gathers (2048 idxs = 129 descs/DMA
  engine) double-buffer inside the per-queue descriptor ring; 32-col
  gathers overflow it and halve throughput.

  Per gather the vector engine computes dot = reduce(g_emb * h). The
  tail over [P,1024] uses host-precomputed per-candidate scalars
  c = tanh^2|x|, d = log(1-c) (shipped dense in sorted order):
      n2 = max(s_h - 2*dot + c, MIN)
      out = sig*d - ln(n2) + (MARGIN + bias_head + (1-sig)*ln(1-s_h))
  (tanh(|x|)/|x| ~ 1 to 2e-5 at this data scale, so dot needs no
  expmap rescale.)
  Host maps (b, n) -> sorted rank and reassembles with take_along_axis.
"""

import numpy as np
import ml_dtypes

import concourse.bacc as bacc
import concourse.bass as bass
import concourse.mybir as mybir
import concourse.tile as tile
from concourse import bass_utils

F32 = mybir.dt.float32
BF16 = mybir.dt.bfloat16
I16 = mybir.dt.int16
AX = mybir.AxisListType
OP = mybir.AluOpType
AF = mybir.ActivationFunctionType

MIN_NORM = 1e-15
MARGIN = 9.0
N_ENT, N_REL, D = 200000, 500, 64
B, NCAND = 1024, 1024
NCORES = 8
P = 128                   # batch rows per core == partitions
EW = 128                  # bf16 elems per table row (256B)

GCH = 16                  # sorted-rank columns per gather
NG = NCAND // GCH         # gathers per core
WIN = 32768               # int16 window rows
NQ_SWDGE = 4
NI = GCH * P              # idxs per gather
IC = NI // 16             # int16 idx columns per gather

# compile-time window base per gather: centered on the mid-column quantile
GBASE = [
    int(np.clip(round(N_ENT * (g * GCH + GCH // 2) / NCAND) - WIN // 2,
                0, N_ENT - WIN))
    for g in range(NG)
]

_CACHE: dict = {}


def _patch_tile_lane_assignment():
    """Make Tile's DMASW completion-lane rotation queue-aware.

    Tile round-robins Pool-engine DMAs over 8 DMASW lanes ignoring the
    SWDGE queue_num; the SWDGE ucode locks each completion sem lane to
    one queue, so multi-queue kernels hit cross-queue lane collisions.
    Give each queue a fixed pair of lanes: queue q -> lanes {2q, 2q+1}.
    """
    import inspect
    import textwrap
    from concourse import tile_sem_assignment as tsa

    if getattr(tsa, "_lane_patch_done", False):
        return
    src = inspect.getsource(tsa.TileClockTick._assign_tick)
    old = """            if engine == mybir.EngineType.Pool:
                inst_proc_idx = PROC_NAME_TO_IDX[f"DMASW{self.next_sw_dma_idx}"]
                self.next_sw_dma_idx = (self.next_sw_dma_idx + 1) % self.swdge_sem_count"""
    new = """            if engine == mybir.EngineType.Pool:
                _q = int(getattr(inst, "queue_num", 0) or 0)
                _cnt = getattr(self, "_q_lane_counter", None)
                if _cnt is None:
                    _cnt = self._q_lane_counter = {}
                _c = _cnt.get(_q, 0)
                _cnt[_q] = _c + 1
                _lane = (2 * _q + (_c % 2)) % self.swdge_sem_count
                inst_proc_idx = PROC_NAME_TO_IDX[f"DMASW{_lane}"]
                self.next_sw_dma_idx = (self.next_sw_dma_idx + 1) % self.swdge_sem_count"""
    assert old in textwrap.dedent(src) or old in src, "tile lane patch anchor missing"
    patched = src.replace(old, new)
    ns = dict(vars(tsa))
    exec(textwrap.dedent(patched), ns)
    tsa.TileClockTick._assign_tick = ns["_assign_tick"]
    tsa._lane_patch_done = True


def _build(with_bias):
    _patch_tile_lane_assignment()
    nc = bacc.Bacc(
        "TRN2",
        target_bir_lowering=False,
        debug=False,
        enable_asserts=False,
        num_devices=NCORES,
        num_swdge_queues=NQ_SWDGE,
    )
    TB = nc.dram_tensor("tab_bf", [N_ENT, EW], BF16, kind="ExternalInput")
    GI = nc.dram_tensor("gidx", [P, NG * IC], I16, kind="ExternalInput")
    HBF = nc.dram_tensor("h_bf", [P, D], BF16, kind="ExternalInput")
    ROWC = nc.dram_tensor("rowc", [P, 3], F32, kind="ExternalInput")  # s_h|sig|c_b
    CA = nc.dram_tensor("c_all", [P, NCAND], BF16, kind="ExternalInput")
    DA = nc.dram_tensor("d_all", [P, NCAND], BF16, kind="ExternalInput")
    BT = (nc.dram_tensor("bt_all", [P, NCAND], BF16, kind="ExternalInput")
          if with_bias else None)
    OUT = nc.dram_tensor("out", [P, NCAND], F32, kind="ExternalOutput")

    with tile.TileContext(nc) as tc:
        with (
            tc.tile_pool(name="small", bufs=1) as sp,
            tc.tile_pool(name="big", bufs=2) as bp,
        ):
            # dummy 128-idx gather issued first: pulls the Q7 gather ucode
            # library load off the critical path (overlaps input DMAs)
            idx0 = sp.tile([P, 8], I16)
            nc.vector.memset(idx0[:], 0)
            scratch = sp.tile([P, EW], BF16)
            nc.gpsimd.dma_gather(
                out_ap=scratch[:].rearrange("p (n d) -> p n d", d=EW),
                in_ap=TB[0:WIN, :],
                idxs_ap=idx0[:],
                num_idxs=128,
                num_idxs_reg=128,
                elem_size=EW,
                single_packet=False,
                queue_num=0,
            )

            hbf = sp.tile([P, D], BF16)
            nc.sync.dma_start(hbf[:], HBF[:])
            rowc = sp.tile([P, 3], F32)
            nc.sync.dma_start(rowc[:], ROWC[:])
            s_h = rowc[:, 0:1]
            sig = rowc[:, 1:2]
            c_b = rowc[:, 2:3]

            out_sb = sp.tile([P, NCAND], F32)
            dot_all = sp.tile([P, NCAND], F32)

            # ---- candidate gathers: fetch + dot only ----
            for g in range(NG):
                q = g % NQ_SWDGE
                ksl = slice(g * GCH, (g + 1) * GCH)
                gidx_t = bp.tile([P, IC], I16, tag="gidx", name=f"gidx{g}", bufs=8)
                nc.sync.dma_start(gidx_t[:], GI[:, g * IC:(g + 1) * IC])
                gt = bp.tile([P, GCH * EW], BF16, tag="g", name=f"g{g}", bufs=8)
                g3 = gt[:].rearrange("p (n d) -> p n d", d=EW)
                nc.gpsimd.dma_gather(
                    out_ap=g3,
                    in_ap=TB[GBASE[g]:GBASE[g] + WIN, :],
                    idxs_ap=gidx_t[:],
                    num_idxs=NI,
                    num_idxs_reg=NI,
                    elem_size=EW,
                    single_packet=False,
                    queue_num=q,
                )
                g64 = g3[:, :, 0:D]
                h_b = hbf[:].rearrange("p (one d) -> p one d", one=1).to_broadcast(
                    [P, GCH, D]
                )
                pr = bp.tile([P, GCH * D], BF16, tag="pr", name=f"pr{g}", bufs=4)
                pr3 = pr[:].rearrange("p (n d) -> p n d", d=D)
                nc.vector.tensor_tensor(pr3, g64, h_b, op=OP.mult)
                nc.vector.tensor_reduce(dot_all[:, ksl], pr3, axis=AX.X, op=OP.add)

            # tail inputs load late on sync so gidx DMAs go first
            c_all = sp.tile([P, NCAND], BF16)
            nc.sync.dma_start(c_all[:], CA[:])
            d_all = sp.tile([P, NCAND], BF16)
            nc.sync.dma_start(d_all[:], DA[:])
            bt_all = None
            if with_bias:
                bt_all = sp.tile([P, NCAND], BF16)
                nc.sync.dma_start(bt_all[:], BT[:])

            # ---- batched tail over [P, NCAND] ----
            n2 = sp.tile([P, NCAND], F32)
            nc.vector.scalar_tensor_tensor(
                n2[:], dot_all[:], -2.0, c_all[:], op0=OP.mult, op1=OP.add
            )
            nc.vector.tensor_scalar_add(n2[:], n2[:], s_h)
            nc.vector.tensor_scalar_max(n2[:], n2[:], MIN_NORM)
            lnum = sp.tile([P, NCAND], F32)
            nc.scalar.activation(lnum[:], n2[:], AF.Ln)
            res = sp.tile([P, NCAND], F32)
            nc.vector.scalar_tensor_tensor(
                res[:], d_all[:], sig, lnum[:], op0=OP.mult, op1=OP.subtract
            )
            if with_bias:
                nc.vector.scalar_tensor_tensor(
                    out_sb[:], res[:], c_b, bt_all[:], op0=OP.add, op1=OP.add
                )
            else:
                nc.vector.tensor_scalar_add(out_sb[:], res[:], c_b)

            nc.sync.dma_start(OUT[:], out_sb[:])

    nc.compile()
    return nc


def get_module(with_bias=False):
    key = ("nc", bool(with_bias))
    if key not in _CACHE:
        _CACHE[key] = _build(bool(with_bias))
    return _CACHE[key]


def _np_head_chain(u_idx, r_idx, emb, rel_diag, rb1, rb2):
    """Transformed heads [B, D] in f64, mirroring the reference chain."""
    def expmap0(u):
        un = np.maximum(np.linalg.norm(u, axis=-1, keepdims=True), MIN_NORM)
        return np.tanh(un) * u / un

    def mobius_add(x, y):
        x2 = np.sum(x * x, -1, keepdims=True)
        y2 = np.sum(y * y, -1, keepdims=True)
        xy = np.sum(x * y, -1, keepdims=True)
        num = (1.0 + 2.0 * xy + y2) * x + (1.0 - x2) * y
        den = 1.0 + 2.0 * xy + x2 * y2
        return num / np.maximum(den, MIN_NORM)

    def givens(r, x):
        g = r.reshape(r.shape[:-1] + (-1, 2))
        g = g / np.maximum(np.linalg.norm(g, axis=-1, keepdims=True), MIN_NORM)
        xp = x.reshape(x.shape[:-1] + (-1, 2))
        out = np.stack(
            [g[..., 0] * xp[..., 0] - g[..., 1] * xp[..., 1],
             g[..., 1] * xp[..., 0] + g[..., 0] * xp[..., 1]], axis=-1)
        return out.reshape(x.shape)

    head = expmap0(emb[u_idx])
    head = mobius_add(head, expmap0(rb1[r_idx]))
    head = givens(rel_diag[r_idx], head)
    head = mobius_add(head, expmap0(rb2[r_idx]))
    return head


def make_in_maps(u_idx, r_idx, v_idx, emb_entity, rel_diag, relation_bias_1,
                 relation_bias_2, bias_head, bias_tail, sigma):
    emb = np.ascontiguousarray(np.asarray(emb_entity, dtype=np.float32))
    bt = np.asarray(bias_tail, dtype=np.float32)
    ui64 = np.asarray(u_idx).astype(np.int64).reshape(B)
    ri64 = np.asarray(r_idx).astype(np.int64).reshape(B)
    vi = np.asarray(v_idx).astype(np.int64).reshape(B, NCAND)

    # per-entity tail scalars in f64: c = tanh^2|x|, d = log(1 - c)
    emb64 = emb.astype(np.float64)
    s = np.sum(emb64 ** 2, axis=1)
    un = np.maximum(np.sqrt(s), MIN_NORM)
    th = np.tanh(un)
    c = th * th
    dcol = np.log(np.maximum(1.0 - c, MIN_NORM))
    tab = np.zeros((N_ENT, EW), dtype=ml_dtypes.bfloat16)
    tab[:, 0:D] = emb.astype(ml_dtypes.bfloat16)

    # host-side head chain + per-row constants
    heads = _np_head_chain(ui64, ri64, emb64,
                           np.asarray(rel_diag, np.float64),
                           np.asarray(relation_bias_1, np.float64),
                           np.asarray(relation_bias_2, np.float64))
    s_h = np.sum(heads * heads, axis=-1)
    sg = 1.0 / (1.0 + np.exp(-np.asarray(sigma, np.float64)[ri64]))
    cb = (MARGIN + np.asarray(bias_head, np.float64)[ui64]
          + (1.0 - sg) * np.log(np.maximum(1.0 - s_h, MIN_NORM)))
    h_bf = heads.astype(ml_dtypes.bfloat16)
    rowc = np.stack([s_h, sg, cb], axis=1).astype(np.float32)   # [B, 3]

    has_bias = bool(np.any(bt))

    order = np.argsort(vi, axis=1, kind="stable")        # [B, NCAND]
    ranks = np.empty_like(order, dtype=np.int64)
    np.put_along_axis(ranks, order, np.arange(NCAND, dtype=np.int64)[None, :], axis=1)
    vs = np.take_along_axis(vi, order, axis=1)           # sorted values

    bases = np.repeat(np.asarray(GBASE, dtype=np.int64), GCH)[None, :]  # [1, NCAND]
    loc = vs - bases                                     # window-local
    viol = (loc < 0) | (loc > WIN - 1)                   # [B, NCAND] on sorted cols
    loc_cl = np.clip(loc, 0, WIN - 1).astype(np.int16)

    c_bf = c.astype(ml_dtypes.bfloat16)
    d_bf = dcol.astype(ml_dtypes.bfloat16)
    bt_bf = bt.astype(ml_dtypes.bfloat16)
    in_maps = []
    aux_ranks = []
    for cidx in range(NCORES):
        sl = slice(cidx * P, (cidx + 1) * P)
        lc = loc_cl[sl]                                  # [P, NCAND] int16
        parts = []
        for g in range(NG):
            st = lc[:, g * GCH:(g + 1) * GCH]            # [P, GCH]
            stream = st.T.ravel()                        # i = c*128 + p
            wrapped = stream.reshape(-1, 16).T           # [16, NI/16]
            parts.append(np.tile(wrapped, (8, 1)))       # [128, NI/16]
        gidx = np.ascontiguousarray(np.concatenate(parts, axis=1))
        assert gidx.shape == (P, NG * IC)
        vs_c = vs[sl]                                    # sorted entity ids
        in_map = {
            "tab_bf": tab,
            "gidx": gidx,
            "h_bf": np.ascontiguousarray(h_bf[sl]),
            "rowc": np.ascontiguousarray(rowc[sl]),
            "c_all": np.ascontiguousarray(c_bf[vs_c]),
            "d_all": np.ascontiguousarray(d_bf[vs_c]),
        }
        if has_bias:
            in_map["bt_all"] = np.ascontiguousarray(bt_bf[vs_c])
        in_maps.append(in_map)
        aux_ranks.append(ranks[sl])

    # exact host fixup values for window-violating candidates
    fix = None
    nviol = int(viol.sum())
    if nviol:
        vb, vc = np.nonzero(viol)                        # batch row, sorted col
        v_ent = vs[vb, vc]                               # entity ids
        hb = heads[vb]                                   # [K, D]
        x = emb64[v_ent]
        unx = np.maximum(np.linalg.norm(x, axis=-1, keepdims=True), MIN_NORM)
        t = np.tanh(unx) * x / unx                       # expmap0(tail)
        n2 = np.sum((hb - t) ** 2, axis=-1)
        s_hb = np.sum(hb * hb, axis=-1)
        s_t = np.sum(t * t, axis=-1)
        d_tail = np.log(np.maximum(n2, MIN_NORM) / np.maximum(1.0 - s_t, MIN_NORM))
        d_head = np.log(np.maximum(n2, MIN_NORM) / np.maximum(1.0 - s_hb, MIN_NORM))
        sgv = sg[vb]
        dist = sgv * d_tail + (1.0 - sgv) * d_head
        val = (MARGIN - dist
               + np.asarray(bias_head, np.float64)[ui64[vb]]
               + np.asarray(bias_tail, np.float64)[v_ent])
        fix = (vb, vc, val.astype(np.float32))
    return in_maps, (aux_ranks, fix), has_bias


def assemble(results, aux):
    aux_ranks, fix = aux
    sorted_scores = np.concatenate(
        [np.asarray(results[c]["out"]) for c in range(NCORES)], axis=0
    )                                                    # [B, NCAND] sorted cols
    if fix is not None:
        vb, vc, val = fix
        sorted_scores[vb, vc] = val
    ranks = np.concatenate(aux_ranks, axis=0)
    return np.take_along_axis(sorted_scores, ranks, axis=1).astype(np.float32)


def kernel(**inputs) -> np.ndarray:
    in_maps, aux, has_bias = make_in_maps(**inputs)
    nc = get_module(has_bias)
    res = bass_utils.run_bass_kernel_spmd(
        nc, in_maps, core_ids=list(range(NCORES))
    )
    return assemble(res.results, aux)
